# revision 2
# baseline (speedup 1.0000x reference)
"""CrossLayerTranscoder Trainium2 kernel v2, 8-core feature-parallel.

vs baseline: encode stays true-fp32 (top-64 selection must match the fp32
reference exactly — bf16/fp8 encode perturbs the near-threshold set and
costs ~8e-2 rel err). Decode runs fp8e4 DoubleRow at 0.5 cyc/row with
error-feedback acts: each activation is stored as an e4m3 pair
(a0 = e4(a), a1 = e4(a - a0)) and the two DR slots compute
W*a0 + W*a1 = W*(a0+a1), recovering ~bf16-level activation precision while
the PE runs at 2x bf16 speed. W_dec is e4m3 (slot-duplicated), loaded once
per (i,j) pair. Top-k is hierarchical: producer-side top-16 per 256-chunk
(3 DVE passes instead of 15 over the full width), 16x smaller AllToAll,
exact top-64 of the 256 survivors on the consumer shard. ReduceScatter
payload is bf16; DMAs are batched wide.
"""
import os
from contextlib import ExitStack

import numpy as np

L = 12          # layers
B = 2048        # batch rows
D = 768         # d_in
FD = 4096       # dict size
OD = 768        # d_out
TOPK = 64
NCORE = 8
FC = FD // NCORE            # 512 local features
BCH = 512                   # matmul moving-dim chunk
NB = B // BCH               # 4
NBT = B // 128              # 16 topk row tiles
KD = D // 128               # 6 encode k-tiles
NFT = FC // 128             # 4 local f-tiles
NOT = OD // 128             # 6 o-tiles
OSH = OD // NCORE           # 96 output rows per core
BSH = B // NCORE            # 256 threshold rows per core
NEG = -3.0e38
PAIRS = [(i, j) for j in range(L) for i in range(j + 1)]   # 78, j-major


def _build_nc(sim=False, no_decode=False, no_encode=False):
    """sim=True: single-core, collectives stripped (TimelineSim timing)."""
    import concourse.bacc as bacc
    import concourse.mybir as mybir
    import concourse.tile as tile

    F32 = mybir.dt.float32
    BF16 = mybir.dt.bfloat16
    FP8 = mybir.dt.float8e4
    RELU = mybir.ActivationFunctionType.Relu
    COPY = mybir.ActivationFunctionType.Copy
    GE = mybir.AluOpType.is_ge
    MUL = mybir.AluOpType.mult
    SUB = mybir.AluOpType.subtract
    ADD = mybir.AluOpType.add
    BYP = mybir.AluOpType.bypass
    DR = mybir.MatmulPerfMode.DoubleRow
    RG = [list(range(NCORE))]

    nc = bacc.Bacc("TRN2", target_bir_lowering=False, debug=False,
                   num_devices=1 if sim else NCORE)

    x_d = nc.dram_tensor("x_t", [L, D, B], F32, kind="ExternalInput").ap()
    we_d = nc.dram_tensor("w_enc_sl", [L, D, FC], F32,
                          kind="ExternalInput").ap()
    be_d = nc.dram_tensor("b_enc_sl", [L, NFT, 128], F32,
                          kind="ExternalInput").ap()
    # per pair p, o-half h: [k, main(fg,s,o384) | corr(f,cs,o384)] e4m3.
    # main slots = (w0[2fg], w0[2fg+1]); corr slots = (w1[f], w0[f]) where
    # w0 = e4(W), w1 = e4(W - w0). Paired with rhs slots (a0, a1) the three
    # DR products give w0*a0 + w1*a0 + w0*a1 (w1*a1 dropped, ~1e-4).
    wd_d = nc.dram_tensor("w_dec_sl", [len(PAIRS), 2, 128, 3072],
                          FP8, kind="ExternalInput").ap()
    bd_d = nc.dram_tensor("b_dec_sh", [L, OSH], F32, kind="ExternalInput").ap()
    id_d = nc.dram_tensor("ident", [128, 128], F32, kind="ExternalInput").ap()
    out_d = nc.dram_tensor("out_shard", [L, OSH, B], F32,
                           kind="ExternalOutput").ap()

    with tile.TileContext(nc) as tc, ExitStack() as ctx:
        sb_const = ctx.enter_context(tc.tile_pool(name="const", bufs=1))
        sb_x = ctx.enter_context(tc.tile_pool(name="xt", bufs=1))
        sb_we = ctx.enter_context(tc.tile_pool(name="we", bufs=1))
        sb_be = ctx.enter_context(tc.tile_pool(name="be", bufs=2))
        sb_pre = ctx.enter_context(tc.tile_pool(name="pre", bufs=4))
        sb_bf = ctx.enter_context(tc.tile_pool(name="bft", bufs=1))
        sb_s1 = ctx.enter_context(tc.tile_pool(name="sc1", bufs=2))
        sb_st = ctx.enter_context(tc.tile_pool(name="st", bufs=2))
        sb_s2 = ctx.enter_context(tc.tile_pool(name="sc2", bufs=2))
        sb_tb = ctx.enter_context(tc.tile_pool(name="tbc", bufs=1))
        sb_mk = ctx.enter_context(tc.tile_pool(name="msk", bufs=1))
        sb_t1 = ctx.enter_context(tc.tile_pool(name="tm1", bufs=1))
        sb_aq = ctx.enter_context(tc.tile_pool(name="aq", bufs=1))
        sb_as = ctx.enter_context(tc.tile_pool(name="actstr", bufs=3))
        sb_wd = ctx.enter_context(tc.tile_pool(name="wd", bufs=25))
        sb_ev = ctx.enter_context(tc.tile_pool(name="ev", bufs=1))
        sb_out = ctx.enter_context(tc.tile_pool(name="outp", bufs=1))
        sb_bd = ctx.enter_context(tc.tile_pool(name="bdec", bufs=2))

        ps_enc = ctx.enter_context(tc.tile_pool(name="psenc", bufs=2,
                                                space="PSUM"))
        ps_tps = ctx.enter_context(tc.tile_pool(name="pstps", bufs=2,
                                                space="PSUM"))
        ps_dec = ctx.enter_context(tc.tile_pool(name="psdec", bufs=1,
                                                space="PSUM"))

        dram = ctx.enter_context(tc.tile_pool(name="dram", bufs=1,
                                              space="DRAM"))

        ident = sb_const.tile([128, 128], F32)
        nc.sync.dma_start(out=ident[:], in_=id_d)

        # internal DRAM buffers
        acts_dr = [dram.tile([NFT, 2, 128, B], FP8, name=f"acts{i}")
                   for i in range(L)]
        pbf_dr = [dram.tile([B, 32], F32, name=f"pbf{i}") for i in range(L)]
        a2a_dr = [dram.tile([NCORE, BSH, 32], F32, name=f"a2a{i}")
                  for i in range(L)]
        tin_dr = [dram.tile([1, BSH], F32, name=f"tin{i}") for i in range(L)]
        tout_dr = [dram.tile([1, B], F32, name=f"tout{i}", addr_space="Shared")
                   for i in range(L)]
        rsin_dr = [dram.tile([OD, B], BF16, name=f"rsin{j}") for j in range(L)]
        rsout_dr = [dram.tile([OSH, B], BF16, name=f"rsout{j}")
                    for j in range(L)]

        def encode_layer(i):
            wt = sb_we.tile([128, KD, FC], F32, name=f"we_{i}", tag="we")
            nc.sync.dma_start(
                out=wt[:],
                in_=we_d[i].rearrange("(k p) f -> p k f", k=KD))
            bt = sb_be.tile([128, NFT], F32, name=f"be_{i}", tag="be")
            nc.sync.dma_start(out=bt[:], in_=be_d[i].rearrange("f p -> p f"))
            pre = [sb_pre.tile([128, B], F32, name=f"pre_{i}_{f}", tag="pre")
                   for f in range(NFT)]
            for b in range(NB):
                xt = sb_x.tile([128, KD, BCH], F32, name=f"x_{i}_{b}",
                               tag="xt")
                nc.sync.dma_start(
                    out=xt[:],
                    in_=x_d[i].rearrange("(k p) b -> p k b",
                                         k=KD)[:, :, b * BCH:(b + 1) * BCH])
                for f in range(NFT):
                    ps = ps_enc.tile([128, BCH], F32, name=f"eps_{i}_{b}_{f}",
                                     tag="eps")
                    for k in range(KD):
                        nc.tensor.matmul(ps[:],
                                         wt[:, k, f * 128:(f + 1) * 128],
                                         xt[:, k, :],
                                         start=(k == 0), stop=(k == KD - 1))
                    nc.scalar.activation(pre[f][:, b * BCH:(b + 1) * BCH],
                                         ps[:], RELU, bias=bt[:, f:f + 1],
                                         scale=1.0)
            return pre

        def topk_layer(i, pre):
            # producer-side: transpose pre -> [b, f], local top-16 per
            # 256-chunk, ship 32 candidates/row; consumer takes exact top-64
            # of the 8*32 survivors for its row shard.
            sc1 = sb_s1.tile([128, NBT, 32], F32, name=f"sc1_{i}", tag="sc1")
            for bt in range(NBT):
                tps = ps_tps.tile([128, FC], F32, name=f"tps_{i}_{bt}",
                                  tag="tps")
                for f in range(NFT):
                    nc.tensor.transpose(
                        tps[:, f * 128:(f + 1) * 128],
                        pre[f][:, bt * 128:(bt + 1) * 128], ident[:])
                bft = sb_bf.tile([128, FC], F32, name=f"bf_{i}_{bt}", tag="bf")
                nc.scalar.activation(bft[:], tps[:], COPY)
                for c in range(2):
                    cs = slice(c * 256, (c + 1) * 256)
                    nc.vector.max(sc1[:, bt, c * 16:c * 16 + 8], bft[:, cs])
                    nc.vector.match_replace(bft[:, cs],
                                            sc1[:, bt, c * 16:c * 16 + 8],
                                            bft[:, cs], NEG)
                    nc.vector.max(sc1[:, bt, c * 16 + 8:c * 16 + 16],
                                  bft[:, cs])
            nc.sync.dma_start(
                out=pbf_dr[i][:].rearrange("(t p) k -> p t k", t=NBT),
                in_=sc1[:])
            if not sim:
                nc.gpsimd.collective_compute(
                    "AllToAll", BYP, replica_groups=RG,
                    ins=[pbf_dr[i][:].opt()], outs=[a2a_dr[i][:].opt()])
            sel_src = (pbf_dr[i][:].rearrange("(r p) k -> r p k", r=NCORE)
                       if sim else a2a_dr[i][:])
            for bt in range(BSH // 128):
                st = sb_st.tile([128, NCORE * 32], F32, name=f"st_{i}_{bt}",
                                tag="st")
                src = sel_src[:, bt * 128:(bt + 1) * 128, :].rearrange(
                    "r p k -> p r k")
                nc.sync.dma_start(out=st[:].rearrange("p (r k) -> p r k",
                                                      r=NCORE), in_=src)
                sc = sb_s2.tile([128, TOPK], F32, name=f"sc_{i}_{bt}",
                                tag="sc")
                for r in range(8):
                    nc.vector.max(sc[:, r * 8:(r + 1) * 8], st[:])
                    if r < 7:
                        nc.vector.match_replace(st[:], sc[:, r * 8:(r + 1) * 8],
                                                st[:], NEG)
                nc.sync.dma_start(out=tin_dr[i][0, bt * 128:(bt + 1) * 128],
                                  in_=sc[:, 63:64])
            if not sim:
                nc.gpsimd.collective_compute(
                    "AllGather", BYP, replica_groups=RG,
                    ins=[tin_dr[i][:].opt()], outs=[tout_dr[i][:].opt()])
            # mask pre with broadcast thresholds, store acts as e4m3 pair
            # (a0, a1 = residual) for the DoubleRow error-feedback decode
            tb = sb_tb.tile([128, B], F32, name=f"tb_{i}", tag="tb")
            nc.sync.dma_start(out=tb[:],
                              in_=tout_dr[i][0:1, :].to_broadcast([128, B]))
            aq = sb_aq.tile([128, NFT, 2, B], FP8, name=f"aq_{i}", tag="aq")
            for f in range(NFT):
                mk = sb_mk.tile([128, B], BF16, name=f"mk_{i}_{f}", tag="mk")
                nc.vector.tensor_tensor(mk[:], pre[f][:], tb[:], GE)
                # a0 = e4(pre * mk)
                nc.vector.tensor_tensor(aq[:, f, 0, :], pre[f][:], mk[:], MUL)
                # t1 = pre - a0  (exact residual before masking)
                t1 = sb_t1.tile([128, B], F32, name=f"t1_{i}_{f}", tag="t1")
                nc.vector.tensor_tensor(t1[:], pre[f][:], aq[:, f, 0, :], SUB)
                # a1 = e4(t1 * mk)
                nc.vector.tensor_tensor(aq[:, f, 1, :], t1[:], mk[:], MUL)
            nc.sync.dma_start(
                out=acts_dr[i][:].rearrange("f s p b -> p (f s) b"),
                in_=aq[:].rearrange("p f s b -> p (f s) b"))

        def decode_layer(j):
            # recon^T[j][o,b] = sum_{i<=j} W_dec[i,j]^T @ (a0[i]+a1[i]), DR fp8
            for half in range(2):
                wts = []
                for i in range(j + 1):
                    p = PAIRS.index((i, j))
                    wt = sb_wd.tile([128, 3072], FP8,
                                    name=f"wt_{j}_{half}_{i}", tag="wt")
                    nc.sync.dma_start(out=wt[:], in_=wd_d[p, half])
                    wts.append(wt)
                for b in range(NB):
                    ps = ps_dec.tile([128, 3 * BCH], F32,
                                     name=f"dps_{j}_{half}_{b}", tag="dps")
                    for i in range(j + 1):
                        at = sb_as.tile([128, NFT * 2, BCH], FP8,
                                        name=f"at_{j}_{half}_{b}_{i}",
                                        tag="at")
                        nc.sync.dma_start(
                            out=at[:],
                            in_=acts_dr[i][:].rearrange(
                                "f s p b -> p (f s) b")[:, :,
                                                        b * BCH:(b + 1) * BCH])
                        atv = at[:].rearrange("p (f s) b -> p f s b", s=2)
                        wt = wts[i]
                        wcorr = wt[:].rearrange(
                            "p (f cs o) -> p f cs o", f=NFT, cs=2)
                        for o3 in range(3):
                            oc = slice(o3 * 128, (o3 + 1) * 128)
                            pc = ps[:, o3 * BCH:(o3 + 1) * BCH]
                            for fg in range(2):
                                # w0[2fg]*a0[2fg] + w0[2fg+1]*a0[2fg+1]
                                nc.tensor.matmul(
                                    pc, wcorr[:, 2 * fg:2 * fg + 2, 1, oc],
                                    atv[:, 2 * fg:2 * fg + 2, 0, :],
                                    start=(i == 0 and fg == 0),
                                    stop=False, perf_mode=DR)
                            for f in range(NFT):
                                # w1[f]*a0[f] + w0[f]*a1[f]
                                nc.tensor.matmul(
                                    pc, wcorr[:, f, :, oc],
                                    atv[:, f, :, :],
                                    start=False,
                                    stop=(i == j and f == NFT - 1),
                                    perf_mode=DR)
                    ev = sb_ev.tile([128, 3 * BCH], BF16,
                                    name=f"ev_{j}_{half}_{b}", tag="ev")
                    nc.scalar.activation(ev[:], ps[:], COPY)
                    nc.sync.dma_start(
                        out=rsin_dr[j][:].rearrange(
                            "(m p) b -> p m b", m=6)[:, half * 3:half * 3 + 3,
                                                     b * BCH:(b + 1) * BCH],
                        in_=ev[:].rearrange("p (m b) -> p m b", m=3))
            if not sim:
                nc.gpsimd.collective_compute(
                    "ReduceScatter", ADD, replica_groups=RG,
                    ins=[rsin_dr[j][:].opt()], outs=[rsout_dr[j][:].opt()])
            ot = sb_out.tile([OSH, B], BF16, name=f"ot_{j}", tag="ot")
            nc.sync.dma_start(out=ot[:], in_=(rsin_dr[j][0:OSH, :] if sim
                                              else rsout_dr[j][:]))
            bdt = sb_bd.tile([OSH, 1], F32, name=f"bd_{j}", tag="bd")
            nc.sync.dma_start(out=bdt[:], in_=bd_d[j, :][:, None])
            oo = sb_out.tile([OSH, B], F32, name=f"oo_{j}", tag="oo")
            nc.vector.tensor_scalar(oo[:], ot[:], bdt[:], None, ADD)
            nc.sync.dma_start(out=out_d[j], in_=oo[:])

        for lyr in range(L):
            if not no_encode:
                pre = encode_layer(lyr)
                topk_layer(lyr, pre)
            if not no_decode:
                decode_layer(lyr)

    nc.compile()
    return nc


_NC_CACHE = None


def kernel(**inputs) -> np.ndarray:
    global _NC_CACHE
    from concourse.bass_utils import run_bass_kernel_spmd

    import ml_dtypes

    E4 = ml_dtypes.float8_e4m3fn

    x = np.ascontiguousarray(inputs["inputs"])          # [L, B, D]
    W_enc = np.ascontiguousarray(inputs["W_enc"])       # [L, D, FD]
    b_enc = np.ascontiguousarray(inputs["b_enc"])       # [L, FD]
    W_dec = np.ascontiguousarray(inputs["W_dec"])       # [L, L, FD, OD]
    b_dec = np.ascontiguousarray(inputs["b_dec"])       # [L, OD]

    x_t = np.ascontiguousarray(x.transpose(0, 2, 1))    # [L, D, B] fp32
    ident = np.eye(128, dtype=np.float32)

    in_maps = []
    for c in range(NCORE):
        fs = slice(c * FC, (c + 1) * FC)
        be = np.ascontiguousarray(b_enc[:, fs]).reshape(L, NFT, 128)
        # W_dec e4m3 error-feedback layout, per (pair, o-half):
        #   main block [k, fg, s, o384]: slots (w0[2fg], w0[2fg+1])
        #   corr block [k, f, cs, o384]: slots (w1[f], w0[f])
        wfull = np.stack([W_dec[i, j, fs, :] for (i, j) in PAIRS])  # [78,512,768]
        w0 = wfull.astype(E4)
        w1 = (wfull - w0.astype(np.float32)).astype(E4)
        w0h = w0.reshape(len(PAIRS), NFT, 128, 2, 384)   # [p,f,k,h,o]
        w1h = w1.reshape(len(PAIRS), NFT, 128, 2, 384)
        # corr: [p,h,k,f,cs,o] with cs = (w1, w0); the "main" w0
        # slots are sliced out of this same block at cs=1 with f-stride 2
        corr = np.stack([w1h, w0h], axis=4)                # [p,f,k,h,cs,o]
        corr = corr.transpose(0, 3, 2, 1, 4, 5)            # [p,h,k,f,cs,o]
        wd = np.ascontiguousarray(
            corr.reshape(len(PAIRS), 2, 128, 3072))
        in_maps.append({
            "x_t": x_t,
            "w_enc_sl": np.ascontiguousarray(W_enc[:, :, fs]),
            "b_enc_sl": np.ascontiguousarray(be).astype(np.float32),
            "w_dec_sl": wd,
            "b_dec_sh": np.ascontiguousarray(
                b_dec[:, c * OSH:(c + 1) * OSH]),
            "ident": ident,
        })

    if _NC_CACHE is None:
        _NC_CACHE = _build_nc()
    nc = _NC_CACHE

    trace = os.environ.get("KERNEL_TRACE", "0") == "1"
    try:
        res = run_bass_kernel_spmd(nc, in_maps, core_ids=list(range(NCORE)),
                                   trace=trace)
    except ModuleNotFoundError:
        res = run_bass_kernel_spmd(nc, in_maps, core_ids=list(range(NCORE)))
    if res.exec_time_ns is not None:
        print(f"HW exec time: {res.exec_time_ns} ns")
        if res.instructions_and_trace is not None:
            print("trace:", res.instructions_and_trace[1])

    full_t = np.concatenate([res.results[c]["out_shard"]
                             for c in range(NCORE)], axis=1)  # [L, OD, B]
    return np.ascontiguousarray(full_t.transpose(0, 2, 1))
